# revision 5
# baseline (speedup 1.0000x reference)
"""TRN2 Bass kernel for nn_DependentLatentModel (HardKuma gate + LSTMCell scan).

Strategy:
- Data-parallel over batch: B=1024 -> 8 cores x Bc=128.
- Time-chunked scan with warm-up forgetting: T=512 split into CHUNKS chunks of C
  steps; each chunk is scanned independently starting W steps early from zero
  state (the LSTM forget gates contract state error to < fp32 noise within W
  steps; warmup inputs for chunk 0 are exact zero-pads which keep the state
  exactly zero). All chunks scan in lockstep as extra batch in the free dim.
- Two pipelined groups per core (independent chunk halves) hide the serial
  chain latency behind engine throughput.
- All matmuls in f32r (full-rate on PE). Gate h-projections are computed
  in-scan from streamed h (no precompute phase, h read once, phase-major).
- Single ACT table set (natural_log_exp): softplus = ln(1+exp(x)),
  1/x = exp(-ln(x)), sigmoid(x) = exp(-ln(1+exp(-x))), tanh(y) = 2*sig(2y)-1
  (the 2x is folded into the g-gate weights on the host).
"""

import os
import sys
import types

import numpy as np

ENC = 768
ZR = 30
BFULL = 1024
T = 512
NCORES = 8
BC = BFULL // NCORES          # 128 batch per core

C = int(os.environ.get("KERN_C", 128))      # chunk length
W = int(os.environ.get("KERN_W", 32))       # warmup steps
STEPS = C + W
CHUNKS = T // C
FD = BC * CHUNKS              # free dim per core across all chunks
G = 2                         # pipelined groups
FDG = FD // G
EPS = 1e-6

_cache = {}


def _ensure_paths():
    try:
        import concourse.bass  # noqa: F401
    except ImportError:
        for p in ("/opt/trn_rl_repo", "/root/.axon_site/_ro/trn_rl_repo"):
            if os.path.isdir(p) and p not in sys.path:
                sys.path.insert(0, p)


def _ensure_ntff_hook():
    """Register the axon NTFF profile hook if the image's antenv lacks it."""
    try:
        import antenv.axon_hooks  # noqa: F401
        return
    except ImportError:
        pass
    mod = types.ModuleType("antenv.axon_hooks")
    holder = [None]
    mod.set_axon_ntff_profile_hook = lambda h: holder.__setitem__(0, h)
    mod.get_axon_ntff_profile_hook = lambda: holder[0]
    sys.modules["antenv.axon_hooks"] = mod
    try:
        from trn_agent_boot.trn_boot import _ntff_profile_via_ctypes
        hook = _ntff_profile_via_ctypes('/opt/axon/libaxon_pjrt.so')
        if hook is not None:
            mod.set_axon_ntff_profile_hook(hook)
    except Exception:
        pass


def _split_waits(nc, mybir, limit=1):
    """This walrus build allows at most one sync wait per instruction; move
    excess waits onto preceding same-engine NOPs."""
    for fn in nc.m.functions:
        for bb in fn.blocks:
            insts = list(bb.instructions)
            new = []
            changed = False
            ctr = 0
            for inst in insts:
                si = inst.sync_info
                if si is not None and len(si.on_wait) > limit:
                    waits = list(si.on_wait)
                    keep = waits[:limit]
                    excess = waits[limit:]
                    for i0 in range(0, len(excess), limit):
                        nop = mybir.InstNoOp(
                            name=f"{inst.name}-ws{ctr}",
                            sync_info=mybir.SyncInfo(
                                on_wait=excess[i0:i0 + limit], on_update=[]),
                            engine=inst.engine,
                            bass_nofuse=True,
                        )
                        ctr += 1
                        new.append(nop)
                    inst.sync_info = mybir.SyncInfo(
                        on_wait=keep, on_update=list(si.on_update))
                    changed = True
                new.append(inst)
            if changed:
                bb.instructions = new


def _build_module():
    import concourse.bass as bass
    import concourse.mybir as mybir
    from concourse import tile

    f32 = mybir.dt.float32
    f32r = mybir.dt.float32r
    AF = mybir.ActivationFunctionType
    ALU = mybir.AluOpType

    nc = bass.Bass()
    h_pm = nc.declare_dram_parameter("h_pm", [STEPS, ENC, FD], f32r, isOutput=False)
    um_pm = nc.declare_dram_parameter("um_pm", [STEPS, 2, FD], f32, isOutput=False)
    w_g = nc.declare_dram_parameter("w_g", [ENC, 126], f32r, isOutput=False)
    w_ab = nc.declare_dram_parameter("w_ab", [ENC, 33], f32r, isOutput=False)
    w_hh = nc.declare_dram_parameter("w_hh", [33, 126], f32r, isOutput=False)
    w_abhs = nc.declare_dram_parameter("w_abhs", [31, 33], f32r, isOutput=False)
    ones_r = nc.declare_dram_parameter("ones_r", [1, FD], f32r, isOutput=False)
    zout = nc.declare_dram_parameter("zout", [STEPS, FD], f32, isOutput=True)

    # internal DRAM: lu/mask interleaved per phase ([STEPS, 2, FD]; row0=lu, row1=mask)
    lm_d = nc.dram_tensor("lm_d", [STEPS, 2, FD], f32)

    with tile.TileContext(nc) as tc:
        with tc.tile_pool(name="w", bufs=1) as wp, \
             tc.tile_pool(name="st", bufs=1) as stp, \
             tc.tile_pool(name="h", bufs=2) as hp, \
             tc.tile_pool(name="io", bufs=3) as iop, \
             tc.tile_pool(name="zm", bufs=2) as zmp, \
             tc.tile_pool(name="scr", bufs=1) as scr, \
             tc.tile_pool(name="pg", bufs=2, space="PSUM") as pgp, \
             tc.tile_pool(name="pa", bufs=2, space="PSUM") as pap:

            # ---- weights -> SBUF
            wg_t = []
            wab_t = []
            for kc in range(6):
                wt = wp.tile([128, 126], f32r, tag=f"wg{kc}")
                nc.sync.dma_start(out=wt[:], in_=w_g[kc * 128:(kc + 1) * 128, :])
                wg_t.append(wt)
                at = wp.tile([128, 33], f32r, tag=f"wab{kc}")
                nc.sync.dma_start(out=at[:], in_=w_ab[kc * 128:(kc + 1) * 128, :])
                wab_t.append(at)
            whh_t = wp.tile([33, 126], f32r, tag="whh")
            nc.sync.dma_start(out=whh_t[:], in_=w_hh[:])
            wabhs_t = wp.tile([31, 33], f32r, tag="wabhs")
            nc.sync.dma_start(out=wabhs_t[:], in_=w_abhs[:])

            # ---- prologue: lu = ln(1 - clip(u, EPS, 1-EPS)); lm_d[:,0]=lu, [:,1]=mask
            for p0, nrow in ((0, 128), (128, STEPS - 128)) if STEPS > 128 else ((0, STEPS),):
                ut = scr.tile([128, FD], f32, tag="prolog_u")
                nc.sync.dma_start(out=ut[0:nrow, :], in_=um_pm[p0:p0 + nrow, 0, :])
                nc.gpsimd.tensor_scalar(out=ut[0:nrow, :], in0=ut[0:nrow, :],
                                        scalar1=EPS, scalar2=1.0 - EPS,
                                        op0=ALU.max, op1=ALU.min)
                nc.scalar.activation(out=ut[0:nrow, :], in_=ut[0:nrow, :],
                                     func=AF.Ln, scale=-1.0, bias=1.0)
                nc.sync.dma_start(out=lm_d[p0:p0 + nrow, 1, :], in_=ut[0:nrow, :])
            # copy mask rows DRAM->DRAM
            nc.sync.dma_start(out=lm_d[:, 0, :], in_=um_pm[:, 1, :])

            # ---- state init (per group)
            sts = []
            css = []
            for g in range(G):
                st = stp.tile([33, FDG], f32r, tag=f"st{g}")
                cs = stp.tile([30, FDG], f32, tag=f"cs{g}")
                nc.vector.memset(st[:].bitcast(f32), 0.0)
                nc.vector.memset(cs[:], 0.0)
                # row 30 <- 1.0 (bias row)
                nc.sync.dma_start(out=st[30:31, :], in_=ones_r[0:1, g * FDG:(g + 1) * FDG])
                sts.append(st)
                css.append(cs)

            # ---- scan
            nsteps = int(os.environ.get("KERN_STEPS_DEBUG", STEPS))
            for p in range(nsteps):
                for g in range(G):
                    st = sts[g]
                    cs = css[g]
                    gsl = slice(g * FDG, (g + 1) * FDG)

                    ht = hp.tile([128, 6, FDG], f32r, tag=f"h{g}")
                    nc.sync.dma_start(out=ht[:], in_=h_pm[p, :, gsl].rearrange(
                        "(kc kp) f -> kp kc f", kp=128))
                    lm = iop.tile([33, FDG], f32, tag=f"lm{g}")
                    nc.sync.dma_start(out=lm[0:33:32, :], in_=lm_d[p, :, gsl])

                    aps = pap.tile([33, FDG], f32, tag=f"aps{g}")
                    gps = pgp.tile([126, FDG], f32, tag=f"gps{g}")
                    for kc in range(6):
                        nc.tensor.matmul(out=aps[:], lhsT=wab_t[kc][:],
                                         rhs=ht[:, kc, :], start=(kc == 0), stop=False)
                    nc.tensor.matmul(out=aps[:], lhsT=wabhs_t[:], rhs=st[0:31],
                                     start=False, stop=True)
                    for kc in range(6):
                        nc.tensor.matmul(out=gps[:], lhsT=wg_t[kc][:],
                                         rhs=ht[:, kc, :], start=(kc == 0), stop=False)

                    # --- z-chain (ab rows at partitions 0 (a) and 32 (b))
                    A1 = scr.tile([33, FDG], f32, tag=f"A1{g}")
                    nc.scalar.activation(out=A1[:], in_=aps[:], func=AF.Exp)
                    AB = scr.tile([33, FDG], f32, tag=f"AB{g}")
                    nc.scalar.activation(out=AB[:], in_=A1[:], func=AF.Ln, bias=1.0)
                    LAB = scr.tile([33, FDG], f32, tag=f"LAB{g}")
                    nc.scalar.activation(out=LAB[:], in_=AB[:], func=AF.Ln)
                    RAB = scr.tile([33, FDG], f32, tag=f"RAB{g}")
                    nc.scalar.activation(out=RAB[:], in_=LAB[:], func=AF.Exp, scale=-1.0)
                    t1 = scr.tile([1, FDG], f32, tag=f"t1{g}")
                    nc.vector.tensor_tensor(out=t1[:], in0=lm[32:33, :], in1=RAB[32:33, :],
                                            op=ALU.mult)
                    e1 = scr.tile([1, FDG], f32, tag=f"e1{g}")
                    nc.scalar.activation(out=e1[:], in_=t1[:], func=AF.Exp)
                    l1 = scr.tile([1, FDG], f32, tag=f"l1{g}")
                    nc.scalar.activation(out=l1[:], in_=e1[:], func=AF.Ln,
                                         scale=-1.0, bias=1.0)
                    t2 = scr.tile([1, FDG], f32, tag=f"t2{g}")
                    nc.vector.tensor_tensor(out=t2[:], in0=l1[:], in1=RAB[0:1, :],
                                            op=ALU.mult)
                    kk = scr.tile([1, FDG], f32, tag=f"kk{g}")
                    nc.scalar.activation(out=kk[:], in_=t2[:], func=AF.Exp)
                    z0 = scr.tile([1, FDG], f32, tag=f"z0{g}")
                    nc.gpsimd.tensor_scalar(out=z0[:], in0=kk[:], scalar1=1.2,
                                            scalar2=-0.1, op0=ALU.mult, op1=ALU.add)
                    z1 = scr.tile([1, FDG], f32, tag=f"z1{g}")
                    nc.gpsimd.tensor_scalar(out=z1[:], in0=z0[:], scalar1=0.0,
                                            scalar2=1.0, op0=ALU.max, op1=ALU.min)
                    nc.gpsimd.tensor_copy(out=st[32:33, :], in_=z1[:])
                    zm = zmp.tile([1, FDG], f32, tag=f"zmt{g}")
                    nc.gpsimd.tensor_tensor(out=zm[:], in0=z1[:], in1=lm[0:1, :],
                                            op=ALU.mult)
                    nc.sync.dma_start(out=zout[p:p + 1, gsl], in_=zm[0:1, :])

                    # --- gates (waits on z via st row 32)
                    nc.tensor.matmul(out=gps[:], lhsT=whh_t[:], rhs=st[0:33],
                                     start=False, stop=True)
                    EGt = scr.tile([126, FDG], f32, tag=f"EG{g}")
                    nc.scalar.activation(out=EGt[:], in_=gps[:], func=AF.Exp, scale=-1.0)
                    LGt = scr.tile([126, FDG], f32, tag=f"LG{g}")
                    nc.scalar.activation(out=LGt[:], in_=EGt[:], func=AF.Ln, bias=1.0)
                    SGt = scr.tile([126, FDG], f32, tag=f"SG{g}")
                    nc.scalar.activation(out=SGt[:], in_=LGt[:], func=AF.Exp, scale=-1.0)
                    # rows: f@0, i@32, o@64, g@96
                    tg = scr.tile([62, FDG], f32, tag=f"tg{g}")
                    nc.vector.tensor_scalar(out=tg[32:62, :], in0=SGt[96:126, :], scalar1=2.0,
                                            scalar2=-1.0, op0=ALU.mult, op1=ALU.add)
                    Pt = scr.tile([30, FDG], f32, tag=f"P{g}")
                    nc.vector.tensor_tensor(out=Pt[:], in0=tg[32:62, :], in1=SGt[32:62, :],
                                            op=ALU.mult)
                    m1 = scr.tile([30, FDG], f32, tag=f"m1{g}")
                    nc.vector.tensor_tensor(out=m1[:], in0=SGt[0:30, :], in1=cs[:],
                                            op=ALU.mult)
                    nc.vector.tensor_tensor(out=cs[:], in0=m1[:], in1=Pt[:], op=ALU.add)
                    ECt = scr.tile([30, FDG], f32, tag=f"EC{g}")
                    nc.scalar.activation(out=ECt[:], in_=cs[:], func=AF.Exp, scale=-2.0)
                    LCt = scr.tile([30, FDG], f32, tag=f"LC{g}")
                    nc.scalar.activation(out=LCt[:], in_=ECt[:], func=AF.Ln, bias=1.0)
                    SCt = scr.tile([30, FDG], f32, tag=f"SC{g}")
                    nc.scalar.activation(out=SCt[:], in_=LCt[:], func=AF.Exp, scale=-1.0)
                    tc2 = scr.tile([94, FDG], f32, tag=f"tc{g}")
                    nc.vector.tensor_scalar(out=tc2[64:94, :], in0=SCt[:], scalar1=2.0,
                                            scalar2=-1.0, op0=ALU.mult, op1=ALU.add)
                    nc.vector.tensor_tensor(out=st[0:30, :], in0=tc2[64:94, :],
                                            in1=SGt[64:94, :], op=ALU.mult)

    _split_waits(nc, mybir)
    return nc


# destination gate order [f, i, o, g] at padded row blocks 0/32/64/96;
# torch source order is [i, f, g, o]
_SRC_BLOCK = {"i": 0, "f": 1, "g": 2, "o": 3}
_DST = [("f", 0), ("i", 32), ("o", 64), ("g", 96)]


def _pack_gate_cols(Wsrc, ncols_out, row_axis_len):
    """Map [4*ZR, K] torch-gate-ordered weights to [K, 126] lhsT layout with
    quadrant-padded gate blocks; g-gate rows are doubled (tanh(x)=2*sig(2x)-1)."""
    out = np.zeros((row_axis_len, 126), dtype=np.float32)
    for gname, dst0 in _DST:
        s0 = _SRC_BLOCK[gname] * ZR
        blk = Wsrc[s0:s0 + ZR, :].T.astype(np.float32)  # [K, 30]
        if gname == "g":
            blk = blk * 2.0
        out[:, dst0:dst0 + ZR] = blk
    return out


def _pack_gate_vec(vsrc):
    out = np.zeros((126,), dtype=np.float32)
    for gname, dst0 in _DST:
        s0 = _SRC_BLOCK[gname] * ZR
        blk = vsrc[s0:s0 + ZR].astype(np.float32)
        if gname == "g":
            blk = blk * 2.0
        out[dst0:dst0 + ZR] = blk
    return out


def kernel(h, mask, u, Wa, ba, Wb, bb, W_ih, b_ih, W_hh, b_hh):
    _ensure_paths()
    _ensure_ntff_hook()
    from concourse.bass_utils import run_bass_kernel_spmd

    h = np.asarray(h, dtype=np.float32)
    mask_f = np.asarray(mask).astype(np.float32)
    u = np.asarray(u, dtype=np.float32)
    Wa = np.asarray(Wa, dtype=np.float32)
    Wb = np.asarray(Wb, dtype=np.float32)
    ba = np.asarray(ba, dtype=np.float32)
    bb = np.asarray(bb, dtype=np.float32)
    W_ih = np.asarray(W_ih, dtype=np.float32)
    b_ih = np.asarray(b_ih, dtype=np.float32)
    W_hh = np.asarray(W_hh, dtype=np.float32)
    b_hh = np.asarray(b_hh, dtype=np.float32)

    # ---- shared weight packing
    w_g = np.ascontiguousarray(_pack_gate_cols(W_ih[:, :ENC], 126, ENC))  # [768,126]
    w_ab = np.zeros((ENC, 33), dtype=np.float32)
    w_ab[:, 0] = Wa[:ENC, 0]
    w_ab[:, 32] = Wb[:ENC, 0]
    w_hh = np.zeros((33, 126), dtype=np.float32)
    w_hh[0:ZR, :] = _pack_gate_cols(W_hh, 126, ZR)
    w_hh[30, :] = _pack_gate_vec(b_ih + b_hh)          # bias via const-1 row
    w_hh[32, :] = _pack_gate_vec(W_ih[:, ENC])         # z column
    w_abhs = np.zeros((31, 33), dtype=np.float32)
    w_abhs[0:ZR, 0] = Wa[ENC:, 0]
    w_abhs[0:ZR, 32] = Wb[ENC:, 0]
    w_abhs[30, 0] = ba[0]
    w_abhs[30, 32] = bb[0]
    ones_r = np.ones((1, FD), dtype=np.float32)

    # ---- per-core phase-major packing
    in_maps = []
    for c in range(NCORES):
        bsl = slice(c * BC, (c + 1) * BC)
        hc = h[bsl]                                    # [BC, T, ENC]
        ht = np.ascontiguousarray(hc.transpose(1, 2, 0))   # [T, ENC, BC]
        h_pm = np.zeros((STEPS, ENC, FD), dtype=np.float32)
        um_pm = np.zeros((STEPS, 2, FD), dtype=np.float32)
        um_pm[:, 1, :] = 1.0
        uc = u[bsl, :, 0].T                            # [T, BC]
        mc = mask_f[bsl].T                             # [T, BC]
        for j in range(CHUNKS):
            t0 = j * C - W
            p0 = max(0, -t0)
            csl = slice(j * BC, (j + 1) * BC)
            h_pm[p0:, :, csl] = ht[t0 + p0:t0 + STEPS]
            um_pm[p0:, 0, csl] = uc[t0 + p0:t0 + STEPS]
            um_pm[W:, 1, csl] = mc[j * C:(j + 1) * C]
        in_maps.append({
            "h_pm": h_pm, "um_pm": um_pm, "w_g": w_g, "w_ab": w_ab,
            "w_hh": w_hh, "w_abhs": w_abhs, "ones_r": ones_r,
        })

    if "nc" not in _cache:
        _cache["nc"] = _build_module()
    nc = _cache["nc"]

    res = run_bass_kernel_spmd(nc, in_maps, list(range(NCORES)),
                               trace=bool(int(os.environ.get("KERN_TRACE", "0"))))
    _cache["last_result"] = res

    z = np.empty((BFULL, T), dtype=np.float32)
    for c in range(NCORES):
        zo = res.results[c]["zout"]                    # [STEPS, FD]
        for j in range(CHUNKS):
            z[c * BC:(c + 1) * BC, j * C:(j + 1) * C] = \
                zo[W:W + C, j * BC:(j + 1) * BC].T
    return z
